# revision 11
# baseline (speedup 1.0000x reference)
"""GCN layer kernel for 8 Trainium2 NeuronCores.

Computes: out = relu(dinv[:,None] * ((adj+I).T @ (dinv[:,None] * (x@W))) + b)
where dinv = rsqrt(colsum(adj) + 1).

Sharding: adj by COLUMNS across the 8 cores; column block c (with the full
scaled source features z) produces output rows [c*2048, (c+1)*2048) with no
collectives. Host folds the self-loop (+I) into the block and casts it to
fp8e4 ({0,1,2} exact) -- HBM traffic is the roofline (1 GB adjacency vs
64-wide features).

Modes:
  fp8pair  baseline: bf16 z stationary, 2x column-group tiled matmuls
           (tile_position) so two 64-wide stationaries compute concurrently;
           dinv+bias+relu on device.
  dr       DoubleRow fp8: stationary is [z8 | r8] (value + residual
           correction, both e4m3) = 128 wide, perf_mode=DoubleRow contracts
           2 k-tiles per matmul at 0.5 cyc/row -> 2x PE throughput vs
           fp8pair in theory (equal in practice: per-MM weight loads
           serialize). Fewer bytes than fp8pair: no on-device dinv table,
           bf16 output. PSUM [128, 2048] is copied to bf16 and DMA'd out;
           dinv/bias/relu and the z+r combine happen on host.
  dmaa     probe: adjacency DMA loads only, no compute.
  mmp      probe: fp8pair matmuls from one resident tile (1/64 DMA traffic).
  mmdr     probe: DoubleRow matmuls from one resident tile.

reps>1 wraps the pass body in a tc.For_i hardware loop (for timing; the
instruction stream stays one-body-sized).
"""

import sys

import numpy as np

if "/opt/trn_rl_repo" not in sys.path:
    sys.path.insert(0, "/opt/trn_rl_repo")

import ml_dtypes

N = 16384
F = 64
NCORES = 8
NB = N // NCORES  # 2048 columns (= output rows) per core
P = 128
KT = N // P  # 128 k-tiles of 128 source rows each
MM_N = 512  # moving free dim per matmul chunk (one f32 PSUM bank)
NCHUNK = NB // MM_N  # 4
DMA_BATCH = 4  # k-tiles per dma_start (must be even for dr)
APOOL_BUFS = 8  # in-flight A-tile DMA slots
ALT_DMA = False  # alternate A DMAs between sync and scalar HWDGE rings
SPLIT_DRAIN = False  # drain dr output per chunk vs one shot
MODE = "dr"

_BASS_CACHE: dict = {}


def _build_bass(reps: int = 1, mode: str | None = None, kb: int | None = None,
                alt_dma: bool | None = None, split_drain: bool = SPLIT_DRAIN,
                apool_bufs: int | None = None):
    if mode is None:
        mode = MODE
    if kb is None:
        kb = DMA_BATCH
    if alt_dma is None:
        alt_dma = ALT_DMA
    if apool_bufs is None:
        apool_bufs = APOOL_BUFS
    key = (reps, mode, kb, apool_bufs, alt_dma, split_drain)
    if key in _BASS_CACHE:
        return _BASS_CACHE[key]

    import concourse.mybir as mybir
    import concourse.tile as tile
    from concourse import bacc

    nc = bacc.Bacc("TRN2", target_bir_lowering=False, debug=False,
                   num_devices=NCORES)

    fp8 = mybir.dt.float8e4
    dr = mode in ("dr", "mmdr")
    pair = mode in ("fp8pair", "mmp")
    resident = mode in ("mmp", "mmdr")

    a_in = nc.dram_tensor("a", [N, NB], fp8, kind="ExternalInput")
    if dr:
        z_in = nc.dram_tensor("z", [P, KT // 2, 2, 2 * F], fp8,
                              kind="ExternalInput")
        o_out = nc.dram_tensor("o", [P, NB], mybir.dt.bfloat16,
                               kind="ExternalOutput")
    else:
        z_in = nc.dram_tensor("z", [P, KT * F], mybir.dt.bfloat16,
                              kind="ExternalInput")
        o_out = nc.dram_tensor("o", [2 * F, NB], mybir.dt.float32,
                               kind="ExternalOutput")
        b_in = nc.dram_tensor("bvec", [2 * F, 1], mybir.dt.float32,
                              kind="ExternalInput")
        d_in = nc.dram_tensor("dinv", [2 * F, NB], mybir.dt.float32,
                              kind="ExternalInput")

    # [KT/kb, 128, kb, NB]: kb consecutive k-tiles per DMA
    a_tiles = a_in.ap().rearrange("(g t p) i -> g p t i", t=kb, p=P)
    relu = mybir.ActivationFunctionType.Relu

    with tile.TileContext(nc) as tc:
        with (
            tc.tile_pool(name="singles", bufs=1) as singles,
            tc.tile_pool(name="apool", bufs=apool_bufs) as apool,
            tc.tile_pool(name="psum", bufs=1, space="PSUM") as psum_pool,
        ):
            if dr:
                z_sb = singles.tile([P, KT // 2, 2, 2 * F], fp8)
            else:
                z_sb = singles.tile([P, KT * F], mybir.dt.bfloat16)
            nc.sync.dma_start(z_sb[:], z_in.ap())
            if not dr:
                b_sb = singles.tile([2 * F, 1], mybir.dt.float32)
                nc.sync.dma_start(b_sb[:], b_in.ap())
                d_sb = singles.tile([2 * F, NB], mybir.dt.float32, tag="d_sb")
                nc.sync.dma_start(d_sb[:], d_in.ap())

            mm_tile = None
            if resident:
                mm_tile = singles.tile([P, kb, NB], fp8, tag="mm_tile")
                nc.sync.dma_start(mm_tile[:], a_tiles[0])

            def body():
                ps = psum_pool.tile([P, NB], mybir.dt.float32)
                for g in range(KT // kb):
                    if resident:
                        at = mm_tile
                    else:
                        at = apool.tile([P, kb, NB], fp8)
                        eng = nc.scalar if (alt_dma and g % 2) else nc.sync
                        eng.dma_start(at[:], a_tiles[g])
                    if mode == "dmaa":
                        continue
                    if dr:
                        for h in range(kb // 2):  # DoubleRow pair within DMA
                            gp = g * (kb // 2) + h
                            zg = z_sb[:, gp]  # [128, 2, 128]
                            for nn in range(NCHUNK):
                                nc.tensor.matmul(
                                    ps[:, nn * MM_N:(nn + 1) * MM_N],
                                    lhsT=zg,
                                    rhs=at[:, 2 * h:2 * h + 2,
                                           nn * MM_N:(nn + 1) * MM_N],
                                    start=(gp == 0),
                                    stop=(gp == KT // 2 - 1),
                                    perf_mode=mybir.MatmulPerfMode.DoubleRow,
                                )
                    else:
                        for t in range(kb):
                            kt = g * kb + t
                            zk = z_sb[:, kt * F:(kt + 1) * F]
                            for nn in range(NCHUNK):
                                h = nn % 2
                                nc.tensor.matmul(
                                    ps[h * F:(h + 1) * F,
                                       nn * MM_N:(nn + 1) * MM_N],
                                    lhsT=zk,
                                    rhs=at[:, t, nn * MM_N:(nn + 1) * MM_N],
                                    start=(kt == 0),
                                    stop=(kt == KT - 1),
                                    tile_position=(0, h * F),
                                )

                if dr:
                    out_sb = singles.tile([P, NB], mybir.dt.bfloat16,
                                          tag="out_sb")
                    if split_drain:
                        # drain per chunk so the tail pipelines
                        for nn in range(NCHUNK):
                            sf = slice(nn * MM_N, (nn + 1) * MM_N)
                            nc.vector.tensor_copy(out_sb[:, sf], ps[:, sf])
                            nc.sync.dma_start(o_out.ap()[:, sf],
                                              out_sb[:, sf])
                    else:
                        nc.vector.tensor_copy(out_sb[:], ps[:])
                        nc.sync.dma_start(o_out.ap(), out_sb[:])
                    return
                out_sb = singles.tile([P, NB], mybir.dt.float32, tag="out_sb")
                if mode == "dmaa":
                    nc.vector.tensor_copy(out_sb[:F, :F], z_sb[:F, :F])
                else:
                    # touch only the written PSUM quadrants
                    for nn in range(NCHUNK):
                        h = nn % 2
                        sp = slice(h * F, (h + 1) * F)
                        sf = slice(nn * MM_N, (nn + 1) * MM_N)
                        nc.vector.tensor_mul(out_sb[sp, sf], ps[sp, sf],
                                             d_sb[sp, sf])
                        nc.scalar.activation(out_sb[sp, sf], out_sb[sp, sf],
                                             relu, bias=b_sb[sp], scale=1.0)
                nc.sync.dma_start(o_out.ap(), out_sb[:2 * F, :])

            if reps == 1:
                body()
            else:
                with tc.For_i(0, reps, 1):
                    body()

    nc.compile()
    _BASS_CACHE[key] = nc
    return nc


def _host_prep(x, adj, W, b, mode=None):
    """Host-side sharding/preprocessing -> (per-core input maps, post-dict)."""
    if mode is None:
        mode = MODE
    dr = mode in ("dr", "mmdr")
    x = np.asarray(x, dtype=np.float32)
    adj = np.asarray(adj, dtype=np.float32)
    W = np.asarray(W, dtype=np.float32)
    b = np.asarray(b, dtype=np.float32)

    deg = adj.sum(axis=0) + 1.0
    dinv = np.where(deg > 0, 1.0 / np.sqrt(deg), 0.0).astype(np.float32)

    z = (dinv[:, None] * (x @ W)).astype(np.float32)  # [N, F]

    if dr:
        zs = 64.0 * z
        z8 = zs.astype(ml_dtypes.float8_e4m3)
        rs = 16.0 * (zs - z8.astype(np.float32))
        r8 = rs.astype(ml_dtypes.float8_e4m3)
        # zr[kt*128+p, 0:64] = z8 row, [64:128] = r8 row -> [P, KT/2, 2, 128]
        zr = np.concatenate([z8, r8], axis=1)  # [N, 128] fp8
        z_dev = np.ascontiguousarray(
            zr.reshape(KT // 2, 2, P, 2 * F).transpose(2, 0, 1, 3)
        )
    else:
        # k-major layout: z_sb[p, kt*F + f] = z[kt*128 + p, f]
        z_dev = np.ascontiguousarray(
            z.reshape(KT, P, F).transpose(1, 0, 2).reshape(P, KT * F)
        ).astype(ml_dtypes.bfloat16)
        b_dev = np.ascontiguousarray(
            np.concatenate([b, b]).reshape(2 * F, 1))

    in_maps = []
    idx = np.arange(NB)
    for c in range(NCORES):
        cs = c * NB
        blk = adj[:, cs:cs + NB].copy()
        blk[cs + idx, idx] += 1.0  # self-loop (+I); {0,1,2} exact in fp8
        m = {"a": blk.astype(ml_dtypes.float8_e4m3), "z": z_dev}
        if not dr:
            dc = dinv[cs:cs + NB]
            d2 = np.zeros((2 * F, NB), np.float32)
            for nn in range(NCHUNK):
                h = nn % 2
                d2[h * F:(h + 1) * F, nn * MM_N:(nn + 1) * MM_N] = \
                    dc[nn * MM_N:(nn + 1) * MM_N]
            m["bvec"] = b_dev
            m["dinv"] = d2
        in_maps.append(m)
    return in_maps, {"dinv": dinv, "b": b}


def _assemble(results, post, mode=None):
    """Device outputs -> full [N, F] output."""
    if mode is None:
        mode = MODE
    out = np.empty((N, F), dtype=np.float32)
    if mode in ("dr", "mmdr"):
        dinv, b = post["dinv"], post["b"]
        for c in range(NCORES):
            blk = results[c]["o"].astype(np.float32)  # [128, NB] bf16
            agg = blk[:F].T * (1.0 / 64.0) + blk[F:].T * (1.0 / 1024.0)
            cs = c * NB
            out[cs:cs + NB] = np.maximum(
                dinv[cs:cs + NB, None] * agg + b, 0.0)
    else:
        for c in range(NCORES):
            blk = results[c]["o"]  # [128, NB]; chunk nn on rows (nn%2)*64
            cs = c * NB
            for nn in range(NCHUNK):
                h = nn % 2
                out[cs + nn * MM_N:cs + (nn + 1) * MM_N, :] = \
                    blk[h * F:(h + 1) * F, nn * MM_N:(nn + 1) * MM_N].T
    return out


def kernel(x, adj, W, b):
    from concourse import bass_utils

    nc = _build_bass(mode=MODE)
    in_maps, post = _host_prep(x, adj, W, b, mode=MODE)
    res = bass_utils.run_bass_kernel_spmd(nc, in_maps,
                                          core_ids=list(range(NCORES)))
    return _assemble(res.results, post, mode=MODE)


# revision 14
# speedup vs baseline: 1.0019x; 1.0019x over previous
"""GCN layer kernel for 8 Trainium2 NeuronCores.

Computes: out = relu(dinv[:,None] * ((adj+I).T @ (dinv[:,None] * (x@W))) + b)
where dinv = rsqrt(colsum(adj) + 1).

Sharding: adj by COLUMNS across the 8 cores; column block c (with the full
scaled source features z) produces output rows [c*2048, (c+1)*2048) with no
collectives. Host folds the self-loop (+I) into the block and casts it to
fp8e4 ({0,1,2} exact) -- HBM traffic is the roofline (1 GB adjacency vs
64-wide features).

Modes:
  fp8pair  baseline: bf16 z stationary, 2x column-group tiled matmuls
           (tile_position) so two 64-wide stationaries compute concurrently;
           dinv+bias+relu on device.
  dr       DoubleRow fp8: stationary is [z8 | r8] (value + residual
           correction, both e4m3) = 128 wide, perf_mode=DoubleRow contracts
           2 k-tiles per matmul at 0.5 cyc/row -> 2x PE throughput vs
           fp8pair in theory (equal in practice: per-MM weight loads
           serialize). Fewer bytes than fp8pair: no on-device dinv table,
           bf16 output. PSUM [128, 2048] is copied to bf16 and DMA'd out;
           dinv/bias/relu and the z+r combine happen on host.
  dmaa     probe: adjacency DMA loads only, no compute.
  mmp      probe: fp8pair matmuls from one resident tile (1/64 DMA traffic).
  mmdr     probe: DoubleRow matmuls from one resident tile.

reps>1 wraps the pass body in a tc.For_i hardware loop (for timing; the
instruction stream stays one-body-sized).
"""

import sys

import numpy as np

if "/opt/trn_rl_repo" not in sys.path:
    sys.path.insert(0, "/opt/trn_rl_repo")

import ml_dtypes

N = 16384
F = 64
NCORES = 8
NB = N // NCORES  # 2048 columns (= output rows) per core
P = 128
KT = N // P  # 128 k-tiles of 128 source rows each
MM_N = 512  # moving free dim per matmul chunk (one f32 PSUM bank)
NCHUNK = NB // MM_N  # 4
DMA_BATCH = 4  # k-tiles per dma_start (must be even for dr)
APOOL_BUFS = 8  # in-flight A-tile DMA slots
ALT_DMA = True  # alternate A DMAs between sync and scalar HWDGE rings
SPLIT_DRAIN = False  # drain dr output per chunk vs one shot
PACK_A = True  # host pre-packs adjacency into the DMA tile layout
MODE = "dr"

_BASS_CACHE: dict = {}


def _build_bass(reps: int = 1, mode: str | None = None, kb: int | None = None,
                alt_dma: bool | None = None, split_drain: bool = SPLIT_DRAIN,
                apool_bufs: int | None = None):
    if mode is None:
        mode = MODE
    if kb is None:
        kb = DMA_BATCH
    if alt_dma is None:
        alt_dma = ALT_DMA
    if apool_bufs is None:
        apool_bufs = APOOL_BUFS
    key = (reps, mode, kb, apool_bufs, alt_dma, split_drain)
    if key in _BASS_CACHE:
        return _BASS_CACHE[key]

    import concourse.mybir as mybir
    import concourse.tile as tile
    from concourse import bacc

    nc = bacc.Bacc("TRN2", target_bir_lowering=False, debug=False,
                   num_devices=NCORES)

    fp8 = mybir.dt.float8e4
    dr = mode in ("dr", "mmdr")
    pair = mode in ("fp8pair", "mmp")
    resident = mode in ("mmp", "mmdr")

    a_in = nc.dram_tensor("a", [KT // kb, P, kb, NB] if PACK_A
                          else [N, NB], fp8, kind="ExternalInput")
    if dr:
        z_in = nc.dram_tensor("z", [P, KT // 2, 2, 2 * F], fp8,
                              kind="ExternalInput")
        o_out = nc.dram_tensor("o", [P, NB], mybir.dt.bfloat16,
                               kind="ExternalOutput")
    else:
        z_in = nc.dram_tensor("z", [P, KT * F], mybir.dt.bfloat16,
                              kind="ExternalInput")
        o_out = nc.dram_tensor("o", [2 * F, NB], mybir.dt.float32,
                               kind="ExternalOutput")
        b_in = nc.dram_tensor("bvec", [2 * F, 1], mybir.dt.float32,
                              kind="ExternalInput")
        d_in = nc.dram_tensor("dinv", [2 * F, NB], mybir.dt.float32,
                              kind="ExternalInput")

    # [KT/kb, 128, kb, NB]: kb consecutive k-tiles per DMA
    if PACK_A:
        a_tiles = a_in.ap()
    else:
        a_tiles = a_in.ap().rearrange("(g t p) i -> g p t i", t=kb, p=P)
    relu = mybir.ActivationFunctionType.Relu

    with tile.TileContext(nc) as tc:
        with (
            tc.tile_pool(name="singles", bufs=1) as singles,
            tc.tile_pool(name="apool", bufs=apool_bufs) as apool,
            tc.tile_pool(name="psum", bufs=1, space="PSUM") as psum_pool,
        ):
            ZH = 8  # z head pairs loaded before the a-stream
            if dr:
                zh_sb = singles.tile([P, ZH, 2, 2 * F], fp8, tag="zh")
                zb_sb = singles.tile([P, KT // 2 - ZH, 2, 2 * F], fp8,
                                     tag="zb")
                nc.sync.dma_start(zh_sb[:], z_in.ap()[:, :ZH])
                nc.scalar.dma_start(zb_sb[:], z_in.ap()[:, ZH:])
            else:
                z_sb = singles.tile([P, KT * F], mybir.dt.bfloat16)
                nc.sync.dma_start(z_sb[:], z_in.ap())
            if not dr:
                b_sb = singles.tile([2 * F, 1], mybir.dt.float32)
                nc.sync.dma_start(b_sb[:], b_in.ap())
                d_sb = singles.tile([2 * F, NB], mybir.dt.float32, tag="d_sb")
                nc.sync.dma_start(d_sb[:], d_in.ap())

            mm_tile = None
            if resident:
                mm_tile = singles.tile([P, kb, NB], fp8, tag="mm_tile")
                nc.sync.dma_start(mm_tile[:], a_tiles[0])

            def body():
                ps = psum_pool.tile([P, NB], mybir.dt.float32)
                glast = KT // kb - 1
                for g in range(KT // kb):
                    if resident:
                        at = mm_tile
                    else:
                        at = apool.tile([P, kb, NB], fp8)
                        eng = nc.scalar if (alt_dma and g % 2) else nc.sync
                        if g == glast and not resident:
                            eng.dma_start(at[:, :kb // 2], a_tiles[g][:, :kb // 2])
                            eng.dma_start(at[:, kb // 2:], a_tiles[g][:, kb // 2:])
                        else:
                            eng.dma_start(at[:], a_tiles[g])
                    if mode == "dmaa":
                        continue
                    if dr:
                        for h in range(kb // 2):  # DoubleRow pair within DMA
                            gp = g * (kb // 2) + h
                            zg = (zh_sb[:, gp] if gp < ZH
                                  else zb_sb[:, gp - ZH])  # [128, 2, 128]
                            for nn in range(NCHUNK):
                                nc.tensor.matmul(
                                    ps[:, nn * MM_N:(nn + 1) * MM_N],
                                    lhsT=zg,
                                    rhs=at[:, 2 * h:2 * h + 2,
                                           nn * MM_N:(nn + 1) * MM_N],
                                    start=(gp == 0),
                                    stop=(gp == KT // 2 - 1),
                                    perf_mode=mybir.MatmulPerfMode.DoubleRow,
                                )
                    else:
                        for t in range(kb):
                            kt = g * kb + t
                            zk = z_sb[:, kt * F:(kt + 1) * F]
                            for nn in range(NCHUNK):
                                h = nn % 2
                                nc.tensor.matmul(
                                    ps[h * F:(h + 1) * F,
                                       nn * MM_N:(nn + 1) * MM_N],
                                    lhsT=zk,
                                    rhs=at[:, t, nn * MM_N:(nn + 1) * MM_N],
                                    start=(kt == 0),
                                    stop=(kt == KT - 1),
                                    tile_position=(0, h * F),
                                )

                if dr:
                    out_sb = singles.tile([P, NB], mybir.dt.bfloat16,
                                          tag="out_sb")
                    if split_drain:
                        # drain per chunk so the tail pipelines
                        for nn in range(NCHUNK):
                            sf = slice(nn * MM_N, (nn + 1) * MM_N)
                            nc.vector.tensor_copy(out_sb[:, sf], ps[:, sf])
                            nc.sync.dma_start(o_out.ap()[:, sf],
                                              out_sb[:, sf])
                    else:
                        nc.vector.tensor_copy(out_sb[:], ps[:])
                        nc.scalar.dma_start(o_out.ap(), out_sb[:])
                    return
                out_sb = singles.tile([P, NB], mybir.dt.float32, tag="out_sb")
                if mode == "dmaa":
                    nc.vector.tensor_copy(out_sb[:F, :F], z_sb[:F, :F])
                else:
                    # touch only the written PSUM quadrants
                    for nn in range(NCHUNK):
                        h = nn % 2
                        sp = slice(h * F, (h + 1) * F)
                        sf = slice(nn * MM_N, (nn + 1) * MM_N)
                        nc.vector.tensor_mul(out_sb[sp, sf], ps[sp, sf],
                                             d_sb[sp, sf])
                        nc.scalar.activation(out_sb[sp, sf], out_sb[sp, sf],
                                             relu, bias=b_sb[sp], scale=1.0)
                nc.sync.dma_start(o_out.ap(), out_sb[:2 * F, :])

            if reps == 1:
                body()
            else:
                with tc.For_i(0, reps, 1):
                    body()

    nc.compile()
    _BASS_CACHE[key] = nc
    return nc


def _host_prep(x, adj, W, b, mode=None):
    """Host-side sharding/preprocessing -> (per-core input maps, post-dict)."""
    if mode is None:
        mode = MODE
    dr = mode in ("dr", "mmdr")
    x = np.asarray(x, dtype=np.float32)
    adj = np.asarray(adj, dtype=np.float32)
    W = np.asarray(W, dtype=np.float32)
    b = np.asarray(b, dtype=np.float32)

    deg = adj.sum(axis=0) + 1.0
    dinv = np.where(deg > 0, 1.0 / np.sqrt(deg), 0.0).astype(np.float32)

    z = (dinv[:, None] * (x @ W)).astype(np.float32)  # [N, F]

    if dr:
        zs = 64.0 * z
        z8 = zs.astype(ml_dtypes.float8_e4m3)
        rs = 16.0 * (zs - z8.astype(np.float32))
        r8 = rs.astype(ml_dtypes.float8_e4m3)
        # zr[kt*128+p, 0:64] = z8 row, [64:128] = r8 row -> [P, KT/2, 2, 128]
        zr = np.concatenate([z8, r8], axis=1)  # [N, 128] fp8
        z_dev = np.ascontiguousarray(
            zr.reshape(KT // 2, 2, P, 2 * F).transpose(2, 0, 1, 3)
        )
    else:
        # k-major layout: z_sb[p, kt*F + f] = z[kt*128 + p, f]
        z_dev = np.ascontiguousarray(
            z.reshape(KT, P, F).transpose(1, 0, 2).reshape(P, KT * F)
        ).astype(ml_dtypes.bfloat16)
        b_dev = np.ascontiguousarray(
            np.concatenate([b, b]).reshape(2 * F, 1))

    in_maps = []
    idx = np.arange(NB)
    for c in range(NCORES):
        cs = c * NB
        blk = adj[:, cs:cs + NB].copy()
        blk[cs + idx, idx] += 1.0  # self-loop (+I); {0,1,2} exact in fp8
        a8 = blk.astype(ml_dtypes.float8_e4m3)
        if PACK_A:
            kb = DMA_BATCH
            a8 = np.ascontiguousarray(
                a8.reshape(KT // kb, kb, P, NB).transpose(0, 2, 1, 3))
        m = {"a": a8, "z": z_dev}
        if not dr:
            dc = dinv[cs:cs + NB]
            d2 = np.zeros((2 * F, NB), np.float32)
            for nn in range(NCHUNK):
                h = nn % 2
                d2[h * F:(h + 1) * F, nn * MM_N:(nn + 1) * MM_N] = \
                    dc[nn * MM_N:(nn + 1) * MM_N]
            m["bvec"] = b_dev
            m["dinv"] = d2
        in_maps.append(m)
    return in_maps, {"dinv": dinv, "b": b}


def _assemble(results, post, mode=None):
    """Device outputs -> full [N, F] output."""
    if mode is None:
        mode = MODE
    out = np.empty((N, F), dtype=np.float32)
    if mode in ("dr", "mmdr"):
        dinv, b = post["dinv"], post["b"]
        for c in range(NCORES):
            blk = results[c]["o"].astype(np.float32)  # [128, NB] bf16
            agg = blk[:F].T * (1.0 / 64.0) + blk[F:].T * (1.0 / 1024.0)
            cs = c * NB
            out[cs:cs + NB] = np.maximum(
                dinv[cs:cs + NB, None] * agg + b, 0.0)
    else:
        for c in range(NCORES):
            blk = results[c]["o"]  # [128, NB]; chunk nn on rows (nn%2)*64
            cs = c * NB
            for nn in range(NCHUNK):
                h = nn % 2
                out[cs + nn * MM_N:cs + (nn + 1) * MM_N, :] = \
                    blk[h * F:(h + 1) * F, nn * MM_N:(nn + 1) * MM_N].T
    return out


def kernel(x, adj, W, b):
    from concourse import bass_utils

    nc = _build_bass(mode=MODE)
    in_maps, post = _host_prep(x, adj, W, b, mode=MODE)
    res = bass_utils.run_bass_kernel_spmd(nc, in_maps,
                                          core_ids=list(range(NCORES)))
    return _assemble(res.results, post, mode=MODE)
